# revision 6
# baseline (speedup 1.0000x reference)
"""Trainium2 Bass kernel for nn_LogicalAttentionLayer (per-token cross-head attention).

Math (per token t):
  q,k,v = x @ W{q,k,v}.T + b   -> [NH=16, HD=64] per token
  scores[h,g] = (q_h . k_g) / 8 ; attn = softmax_g(scores)
  u[h,:] = sum_g attn[h,g] * v[g,:] ; y = u_flat @ Wo.T + bo

Sharding: data-parallel over the 16384 tokens -> 2048 tokens per core x 8 cores.

Per-core plan (all matmuls in bf16 on PE; per-token attention on DVE):
  - gpsimd cast-DMA x,W -> bf16 DRAM scratch; xbar dma-transpose to x^T, W^T in SBUF
  - projections: psum[t,1024] = sum_ic x^T[ic,t-tile].T @ W^T[ic,:]  (+ bias via
    rank-1 ones matmul), ACT drains psum -> bf16 SBUF
  - scores: per g: DVE mul q*[k_g broadcast] + segmented reduce over d
  - softmax: ACT exp(scale=1/8), DVE sum/recip; normalization folded into final scale
  - attn.v: per g: DVE mul+add accumulate; final scale by 1/denom
  - u -> DRAM -> dma-transpose -> u^T ; y = u^T.T @ Wo^T + bo -> fp32 out
"""

import sys

for p in ("/opt/trn_rl_repo",):
    if p not in sys.path:
        sys.path.insert(0, p)

import numpy as np

import concourse.bass as bass
import concourse.mybir as mybir
from concourse import bacc
from concourse.bass_utils import run_bass_kernel_spmd
from concourse.tile import TileContext

NCORES = 8
B, S, HID, NH, HD = 4, 4096, 1024, 16, 64
T_FULL = B * S
T = T_FULL // NCORES          # 2048 tokens per core
NT = T // 128                 # 16 token tiles
IC = HID // 128               # 8 contraction chunks
BF = mybir.dt.bfloat16
F32 = mybir.dt.float32
AX = mybir.AxisListType
OP = mybir.AluOpType
AF = mybir.ActivationFunctionType

_cached = None


def build_program():
    nc = bacc.Bacc("TRN2")

    x = nc.dram_tensor("x", [T, HID], F32, kind="ExternalInput")
    W = {n: nc.dram_tensor(n, [HID, HID], F32, kind="ExternalInput")
         for n in ("Wq", "Wk", "Wv", "Wo")}
    bias = {n: nc.dram_tensor(n, [HID], F32, kind="ExternalInput")
            for n in ("bq", "bk", "bv", "bo")}
    y = nc.dram_tensor("y", [T, HID], F32, kind="ExternalOutput")

    with TileContext(nc) as tc:
        with (
            tc.tile_pool(name="dram", bufs=1, space="DRAM") as dp,
            tc.tile_pool(name="persist", bufs=1) as pp,
            tc.tile_pool(name="qkv", bufs=2) as qp,
            tc.tile_pool(name="attn", bufs=2) as ap_,
            tc.tile_pool(name="out", bufs=2) as op_,
            tc.tile_pool(name="mmps", bufs=1, space="PSUM") as mmps,
            tc.tile_pool(name="yps", bufs=1, space="PSUM") as yps,
        ):
            # ---------------- prologue: weights ----------------
            # cast fp32 -> bf16 in DRAM via gpsimd casting DMA, then xbar-transpose
            wt = {}
            for n in ("Wq", "Wk", "Wv", "Wo"):
                wbf = dp.tile([HID, HID], BF, name=f"{n}bf")
                nc.gpsimd.dma_start(wbf[:, :], W[n][:, :])
                wtn = pp.tile([128, IC * HID], BF, name=f"wt{n}")
                for ic in range(IC):
                    # in_: [1024 rows(o), 128 cols(i)] -> out: [128 (i), 1024 (o)]
                    nc.sync.dma_start_transpose(
                        wtn[:, ic * HID:(ic + 1) * HID],
                        wbf[:, ic * 128:(ic + 1) * 128])
                wt[n] = wtn

            # bias rows [1, 1024] bf16
            brow = {}
            for n in ("bq", "bk", "bv", "bo"):
                bl = pp.tile([1, HID], F32, name=f"{n}f")
                nc.sync.dma_start(bl[:, :], bias[n].rearrange("(a o) -> a o", a=1))
                bb = pp.tile([1, HID], BF, name=f"{n}b")
                nc.scalar.copy(bb[:, :], bl[:, :])
                brow[n] = bb

            ones_sb = pp.tile([1, 128], BF, name="ones")
            nc.vector.memset(ones_sb[:, :], 1.0)

            # ---------------- prologue: x -> bf16 -> x^T ----------------
            xbf = dp.tile([T, HID], BF, name="xbf")
            nc.gpsimd.dma_start(xbf[:, :], x[:, :])
            xt = pp.tile([128, IC * T], BF, name="xt")
            for ic in range(IC):
                nc.sync.dma_start_transpose(
                    xt[:, ic * T:(ic + 1) * T],
                    xbf[:, ic * 128:(ic + 1) * 128])

            ubf = dp.tile([T, HID], BF, name="ubf")

            # ---------------- main loop over token tiles ----------------
            for ti in range(NT):
                # --- projections q,k,v: psum[t,1024] ---
                ps = {}
                for n in ("Wq", "Wk", "Wv"):
                    p = mmps.tile([128, HID], F32, tag=f"ps{n}", name=f"ps{n}")
                    ps[n] = p
                for n, bn in (("Wq", "bq"), ("Wk", "bk"), ("Wv", "bv")):
                    for h in range(2):
                        sl = slice(h * 512, (h + 1) * 512)
                        nc.tensor.matmul(ps[n][:, sl], ones_sb[:, :],
                                         brow[bn][:, sl], start=True, stop=False)
                for ic in range(IC):
                    lt = xt[:, ic * T + ti * 128: ic * T + (ti + 1) * 128]
                    for n in ("Wq", "Wk", "Wv"):
                        for h in range(2):
                            nc.tensor.matmul(
                                ps[n][:, h * 512:(h + 1) * 512], lt,
                                wt[n][:, ic * HID + h * 512: ic * HID + (h + 1) * 512],
                                start=False, stop=(ic == IC - 1))

                q_sb = qp.tile([128, HID], BF, tag="q", name="q_sb")
                k_sb = qp.tile([128, HID], BF, tag="k", name="k_sb")
                v_sb = qp.tile([128, HID], BF, tag="v", name="v_sb")
                nc.scalar.copy(q_sb[:, :], ps["Wq"][:, :])
                nc.scalar.copy(k_sb[:, :], ps["Wk"][:, :])
                nc.scalar.copy(v_sb[:, :], ps["Wv"][:, :])

                # --- scores[t,h,g] = sum_d q[t,h,d]*k[t,g,d] ---
                s_sb = ap_.tile([128, NH * NH], F32, tag="s", name="s_sb")
                q3 = q_sb.rearrange("p (h d) -> p h d", d=HD)
                s3 = s_sb.rearrange("p (h g) -> p h g", g=NH)
                for g in range(NH):
                    prod = ap_.tile([128, HID], BF, tag="prod", name="prod")
                    kg = (k_sb[:, g * HD:(g + 1) * HD]
                          .unsqueeze(1).broadcast_to([128, NH, HD]))
                    p3 = prod.rearrange("p (h d) -> p h d", d=HD)
                    nc.vector.tensor_tensor(p3, q3, kg, OP.mult)
                    nc.vector.tensor_reduce(s3[:, :, g], p3, AX.X, OP.add)

                # --- softmax over g (scale 1/sqrt(64)=0.125 folded into exp) ---
                e_sb = ap_.tile([128, NH * NH], BF, tag="e", name="e_sb")
                nc.scalar.activation(e_sb[:, :], s_sb[:, :], AF.Exp, scale=0.125)
                e3 = e_sb.rearrange("p (h g) -> p h g", g=NH)
                den = ap_.tile([128, NH], F32, tag="den", name="den")
                nc.vector.tensor_reduce(den[:, :], e3, AX.X, OP.add)
                rec = ap_.tile([128, NH], F32, tag="rec", name="rec")
                nc.vector.reciprocal(rec[:, :], den[:, :])

                # --- u[t,h,d] = sum_g e[t,h,g] * v[t,g,d]  (unnormalized) ---
                u_acc = ap_.tile([128, HID], BF, tag="uacc", name="u_acc")
                u3 = u_acc.rearrange("p (h d) -> p h d", d=HD)
                for g in range(NH):
                    vg = (v_sb[:, g * HD:(g + 1) * HD]
                          .unsqueeze(1).broadcast_to([128, NH, HD]))
                    eg = e3[:, :, g].unsqueeze(2).broadcast_to([128, NH, HD])
                    if g == 0:
                        nc.vector.tensor_tensor(u3, vg, eg, OP.mult)
                    else:
                        pt = ap_.tile([128, HID], BF, tag="pt", name="pt")
                        pt3 = pt.rearrange("p (h d) -> p h d", d=HD)
                        nc.vector.tensor_tensor(pt3, vg, eg, OP.mult)
                        nc.vector.tensor_tensor(u3, u3, pt3, OP.add)

                # normalize by 1/den
                u_bf = ap_.tile([128, HID], BF, tag="ubf", name="u_bf")
                ub3 = u_bf.rearrange("p (h d) -> p h d", d=HD)
                rg = rec.unsqueeze(2).broadcast_to([128, NH, HD])
                nc.vector.tensor_tensor(ub3, u3, rg, OP.mult)

                nc.sync.dma_start(ubf[ti * 128:(ti + 1) * 128, :], u_bf[:, :])

                # --- output projection for each group of 4 tiles ---
                if ti % 4 == 3:
                    gr = ti // 4
                    uT = op_.tile([128, IC * 512], BF, tag="uT", name="uT")
                    for ic in range(IC):
                        nc.sync.dma_start_transpose(
                            uT[:, ic * 512:(ic + 1) * 512],
                            ubf[gr * 512:(gr + 1) * 512, ic * 128:(ic + 1) * 128])
                    for tl in range(4):
                        tj = gr * 4 + tl
                        yp = yps.tile([128, HID], F32, tag="y", name="yp")
                        for h in range(2):
                            sl = slice(h * 512, (h + 1) * 512)
                            nc.tensor.matmul(yp[:, sl], ones_sb[:, :],
                                             brow["bo"][:, sl], start=True, stop=False)
                        for ic in range(IC):
                            lt = uT[:, ic * 512 + tl * 128: ic * 512 + (tl + 1) * 128]
                            for h in range(2):
                                nc.tensor.matmul(
                                    yp[:, h * 512:(h + 1) * 512], lt,
                                    wt["Wo"][:, ic * HID + h * 512: ic * HID + (h + 1) * 512],
                                    start=False, stop=(ic == IC - 1))
                        y_sb = op_.tile([128, HID], F32, tag="ysb", name="y_sb")
                        nc.scalar.copy(y_sb[:, :], yp[:, :])
                        nc.sync.dma_start(y[tj * 128:(tj + 1) * 128, :], y_sb[:, :])

    nc.finalize()
    return nc


def kernel(x, Wq, bq, Wk, bk, Wv, bv, Wo, bo, **_unused):
    global _cached
    x = np.ascontiguousarray(np.asarray(x, dtype=np.float32))
    x2 = x.reshape(T_FULL, HID)

    if _cached is None:
        _cached = build_program()
    nc = _cached

    full = {
        "Wq": np.ascontiguousarray(Wq, dtype=np.float32),
        "Wk": np.ascontiguousarray(Wk, dtype=np.float32),
        "Wv": np.ascontiguousarray(Wv, dtype=np.float32),
        "Wo": np.ascontiguousarray(Wo, dtype=np.float32),
        "bq": np.ascontiguousarray(bq, dtype=np.float32),
        "bk": np.ascontiguousarray(bk, dtype=np.float32),
        "bv": np.ascontiguousarray(bv, dtype=np.float32),
        "bo": np.ascontiguousarray(bo, dtype=np.float32),
    }
    in_maps = []
    for c in range(NCORES):
        m = dict(full)
        m["x"] = np.ascontiguousarray(x2[c * T:(c + 1) * T])
        in_maps.append(m)

    res = run_bass_kernel_spmd(nc, in_maps, core_ids=list(range(NCORES)))
    out = np.concatenate([r["y"] for r in res.results], axis=0)
    return out.reshape(B, S, HID).astype(np.float32)


if __name__ == "__main__":
    rng = np.random.default_rng(0)
    ins = {k: rng.standard_normal(v, dtype=np.float32) * (0.02 if k[0] in "Wb" else 1.0)
           for k, v in [("x", (B, S, HID)), ("Wq", (HID, HID)), ("bq", (HID,)),
                        ("Wk", (HID, HID)), ("bk", (HID,)), ("Wv", (HID, HID)),
                        ("bv", (HID,)), ("Wo", (HID, HID)), ("bo", (HID,))]}
    out = kernel(**ins)
    print(out.shape, out.dtype)
